# revision 21
# baseline (speedup 1.0000x reference)
"""Trainium2 Bass kernel for a 3-layer LIF spiking net (nn_Net_9998683865246).

Reference computation (per timestep t, 500 steps, batch 256):
    cur1 = x_t @ W1.T + b1 ; LIF1(m1)  -> s1   (128 features)
    cur2 = s1 @ W2.T + b2  ; LIF2(m2)  -> s2   (256 features)
    cur3 = s2 @ W3.T + b3  ; LIF3(m3)  -> s3   (20 features)
    out = mean_t(s3)                            [256, 20]
LIF (reset-by-subtract, reset from previous mem):
    m <- beta*m + cur - (m_prev > thr)*thr ; s = (m > thr)

Sharding: data-parallel over batch, 32 samples/core on 8 cores.

Layout: feature-on-partition, batch-on-free. All three layers' membranes are
fused into one [128, 112] state M = [m1(32 cols) | m2(64) | m3(16, packed
2-way across partitions: rows 0:20/32:52 = features for batch halves)], with
layer L2 lagging L1 by 2 time-blocks and L3 by 4, so each serial step is ONE
custom DVE instruction (~243 ns/dependent op on HW; the 576-step chain is the
~140 us floor). The membrane ring is 32 deep (two block-sized halves): each
slot is a full state snapshot, so spike extraction runs ONCE PER BLOCK over
the finished half — no per-step cross-engine sync anywhere.

Engine placement (HW-calibrated: gpsimd tensor ops run ~17 ns/elem on real
silicon — 12x the cost model — so gpsimd only does tiny memsets):
  - DVE: the serial chain + per-block s3 reduce/accumulate (lagged 2 blocks
    so the ACT dependency is always satisfied).
  - ACT: PSUM->SBUF current assembly (bias+scale folded in) + spike
    extraction as Sign(y - bound) in {-1,+1}; the (s'+1)/2 affine map folds
    into matmul scale=0.5, rowsum-adjusted biases, and ssum seeded with +T
    (which also makes the all-zero output exact in f32).
  - PE: all matmuls in bf16 (1 cycle/row at 512 moving cols).
  - DMA: x as bf16 (22.4 MB/core/rep).
  - c2 bias rides a K=1 ones-row matmul so one activation assembles both
    feature halves from a 2-bank psum tile; per-block s3 sums park in a
    [52, 32, 16] tile reduced once at the end (+T added exactly afterward).
Measured: ~87-97 us/iteration (staged baseline: 990 us; gpsimd per-step
extraction + f32 DMA bound). Chain floor: 576 serial DVE steps.
"""
import numpy as np
import ml_dtypes

import concourse.bass as bass
import concourse.mybir as mybir
from concourse import bacc
from concourse.tile import TileContext
from concourse.bass_utils import run_bass_kernel_spmd

# problem shape (hardcoded per harness contract)
B, T, C = 256, 500, 700
F1, F2, F3 = 128, 256, 20
NCORES = 8
NB = B // NCORES          # batch per core = 32
BLK = 16                  # timesteps per block
BLKN = BLK * NB           # matmul moving columns per block = 512
TP = 512                  # padded T
XBLK = TP // BLK          # 32 x-blocks
NBLK = XBLK + 4           # fused blocks (L3 lags by 4)
CP = 768                  # C padded to 6*128 for single-DMA x blocks
NKT = CP // 128           # 6 k-chunks of 128
FCOLS = 112               # fused state columns: 32 m1 | 64 m2 | 16 m3
# m3 is packed 2-way: partitions 0:40 = 20 features x 2 batch-halves,
# cols 96:112 = batch-within-half. 640 useful elems in 640 slots of the
# active region instead of 32 full-width columns — the serial chain op
# shrinks from 128 to 112 free columns.
F3H = 32                  # half-1 base partition (matmul out base must be 0/32/64)
F3S = F3H + F3            # stacked m3 partition extent = 52 (rows 20:32 unused)
NB2 = NB // 2             # batch half = 16
RB = 2 * BLK              # membrane ring depth: two block-contiguous halves

f32 = mybir.dt.float32
bf16 = mybir.dt.bfloat16
AL = mybir.AluOpType

# ---- custom fused DVE op (registered into the concourse custom-op table) ----
# LIF_YSTEP_ANT: y' = (y*s0 + c) - [y > s1]*imm2 — one instruction advances the
# whole fused 3-layer pre-reset membrane state (y = m + BSHIFT) by one step.
from concourse.dve_spec import Spec as _Spec, Src0 as _S0, Src1 as _S1, \
    C0 as _C0, C1 as _C1, C2 as _C2, lower as _dve_lower, \
    _has_src1
from concourse import dve_ops as _dvo
from concourse.dve_uop import DveOpSpec as _DveOpSpec


def _lif_ref(in0, in1, s0, s1, imm2):
    y = in0.astype(np.float32)
    return (y * s0 + in1) - (y > s1).astype(np.float32) * imm2


class _DveOp2x(_dvo.DveOp):
    """DveOp whose 2X_1PORT table slot carries the same uop program as 1x.
    For a pure elementwise body the program is mode-invariant (the 2x
    datapath doubles element throughput, not program semantics), so bf16
    packed operands run at 2 elem/cycle/lane. Falls back identically if the
    engine never engages the mode."""

    def compile(self, ver):
        key = (self.name, ver, "2x")
        if (r := _dvo._COMPILE_CACHE.get(key)) is not None:
            return r
        uops = _dve_lower(self.spec, ver=ver)
        result = _DveOpSpec(
            name=self.name,
            opcode=_dvo.get_dve_sub_opcode(self.name),
            uops=uops,
            uops_2x=list(uops),
            perf_max=1,
            rd1_en=_has_src1(self.spec),
        )
        _dvo._COMPILE_CACHE[key] = result
        return result


LIF_YSTEP_ANT = _DveOp2x(
    "LIF_YSTEP_ANT",
    _Spec(body=(_S0 * _C0 + _S1) - (_S0 > _C1) * _C2, reference=_lif_ref),
    subdim=False,
    uops_sha={"v3": "dfb1f0a941a9301a"},
)

for _op in (LIF_YSTEP_ANT,):
    if _op.name not in _dvo._SUB_OPCODE_FOR_NAME:
        _dvo.OPS.append(_op)
        _dvo._SUB_OPCODE_FOR_NAME[_op.name] = (
            _dvo._CUSTOM_DVE_ROW_BASE + len(_dvo.OPS) - 1)
        _dvo.CUSTOM_DVE_SPECS[_op.name] = _op.spec
assert max(_dvo._SUB_OPCODE_FOR_NAME.values()) < 0x20

BSHIFT = 8.0              # domain shift; small so bf16 state keeps precision


def build_kernel(beta: float, thr: float, repeat: int = 1, skip: str = ""):
    """skip: comma-set of {c1,c2,c3,s3,extract,dma,chain} to omit (ablation)."""
    sk = set(skip.split(",")) if skip else set()
    nc = bacc.Bacc(None, target_bir_lowering=False, debug=False)

    x_in = nc.declare_dram_parameter("x", [CP, TP * NB], bf16, isOutput=False)
    w1t_in = nc.declare_dram_parameter("w1t", [CP, F1], bf16, isOutput=False)
    w2t_in = nc.declare_dram_parameter("w2t", [F1, F2], bf16, isOutput=False)
    w3t_in = nc.declare_dram_parameter("w3t", [F2, F3], bf16, isOutput=False)
    b1_in = nc.declare_dram_parameter("b1", [F1, 1], f32, isOutput=False)
    b2t_in = nc.declare_dram_parameter("b2t", [1, F2], bf16, isOutput=False)
    b3_in = nc.declare_dram_parameter("b3", [F3S, 1], f32, isOutput=False)
    out_d = nc.declare_dram_parameter("out", [F3, NB], f32, isOutput=True)

    bound = thr + BSHIFT
    idle = (1.0 - beta) * BSHIFT

    from contextlib import ExitStack
    with TileContext(nc) as tc, ExitStack() as _es:
        wpool = _es.enter_context(tc.tile_pool(name="wpool", bufs=1))
        xpool = _es.enter_context(tc.tile_pool(name="xpool", bufs=2)) \
            if "dma" not in sk else None
        cpool = _es.enter_context(tc.tile_pool(name="cpool", bufs=3))
        spool = _es.enter_context(tc.tile_pool(name="spool", bufs=3)) \
            if "extract" not in sk else None
        mpool = _es.enter_context(tc.tile_pool(name="mpool", bufs=1))
        rpool = _es.enter_context(tc.tile_pool(name="rpool", bufs=2)) \
            if "s3" not in sk else None
        s3pool = _es.enter_context(tc.tile_pool(name="s3pool", bufs=3)) \
            if "extract" not in sk else None
        pc1p = _es.enter_context(tc.tile_pool(name="pc1", bufs=2, space="PSUM")) \
            if "c1" not in sk else None
        pc2p = _es.enter_context(tc.tile_pool(name="pc2", bufs=2, space="PSUM")) \
            if "c2" not in sk else None
        pc3p = _es.enter_context(tc.tile_pool(name="pc3", bufs=2, space="PSUM")) \
            if "c3" not in sk else None
        if True:
            # ---- static weights/biases ----
            w1t = []
            for i in range(NKT):
                w = wpool.tile([128, F1], bf16, name=f"w1t{i}")
                nc.sync.dma_start(out=w[:], in_=w1t_in[i * 128:(i + 1) * 128, :])
                w1t.append(w)
            w2t = wpool.tile([F1, F2], bf16)
            nc.sync.dma_start(out=w2t[:], in_=w2t_in[:])
            w3ta = wpool.tile([128, F3], bf16)
            w3tb = wpool.tile([128, F3], bf16)
            nc.sync.dma_start(out=w3ta[:], in_=w3t_in[0:128, :])
            nc.sync.dma_start(out=w3tb[:], in_=w3t_in[128:256, :])
            b1 = wpool.tile([F1, 1], f32)
            b3 = wpool.tile([F3S, 1], f32)
            bndn = wpool.tile([128, 1], f32)   # -bound, bias for Sign extracts
            nc.gpsimd.memset(bndn[:], -bound)
            b2t = wpool.tile([1, F2], bf16)    # 2*b2_eff as a K=1 lhsT row
            nc.sync.dma_start(out=b2t[:], in_=b2t_in[:])
            ones_t = wpool.tile([1, BLKN], bf16)
            nc.gpsimd.memset(ones_t[:], 1.0)
            nc.sync.dma_start(out=b1[:], in_=b1_in[:])
            nc.sync.dma_start(out=b3[:], in_=b3_in[:])

            o_tile = mpool.tile([F3S, NB2], f32)
            ssum = mpool.tile([F3S, NB2], f32)
            rts = mpool.tile([F3S, XBLK, NB2], bf16)  # per-block s3' sums

            for rep in range(repeat):
                M = mpool.tile([128, RB, FCOLS], bf16, name=f"M_{rep}", tag="M")
                nc.gpsimd.memset(M[:, RB - 1, :], BSHIFT)

                xt = {}      # x tiles ring, keyed (block % 2)
                cur = {}     # cur-block ring, keyed block -> tile
                stk = {}     # S ring, keyed block -> tile (s1|s2, 96 cols)
                s3k = {}     # s3 transposed ring, keyed block -> tile


                def dma_x(j):
                    t = xpool.tile([128, NKT, BLKN], bf16, name="xblk", tag="xblk")
                    src = x_in[:].rearrange("(i p) n -> p i n", p=128)
                    nc.sync.dma_start(
                        out=t[:], in_=src[:, :, j * BLKN:(j + 1) * BLKN])
                    xt[j % 2] = t

                def prep_c1(j):
                    # cur1 for block j -> cur[j][:, :, 0:32], bias b1
                    if j % 2 not in xt:
                        return
                    p = pc1p.tile([F1, BLKN], f32, name="p_c1", tag="p_c1")
                    xb = xt[j % 2]
                    for i in range(NKT):
                        nc.tensor.matmul(p[:], w1t[i][:], xb[:, i, :],
                                         start=(i == 0), stop=(i == NKT - 1))
                    nc.scalar.activation(
                        cur[j][:, :, 0:32],
                        p[:].rearrange("p (k b) -> p k b", k=BLK),
                        mybir.ActivationFunctionType.Identity,
                        bias=b1[:], scale=1.0)

                def new_curblk(j):
                    if "c1" in sk:
                        return
                    t = cpool.tile([128, BLK, FCOLS], bf16, name="curblk", tag="curblk")
                    cur[j] = t
                    if j < 2:
                        nc.gpsimd.memset(t[:, :, 32:FCOLS], idle)
                    elif j < 4:
                        nc.gpsimd.memset(t[:, :, 96:FCOLS], idle)
                    cur.pop(j - 3, None)

                def prep_c2(j):
                    # cur2 for block j from s1 of S[j-2] -> cur[j][:, :, 32:96].
                    # Bias rides a K=1 ones-row matmul so ONE activation
                    # assembles both feature halves from the 2-bank psum tile.
                    if j - 2 not in stk:
                        return
                    s = stk[j - 2]
                    rhs = s[:, :, 0:32]
                    p2 = pc2p.tile([128, 2, BLKN], f32, name="p_c2", tag="p_c2")
                    for g in range(2):
                        nc.tensor.matmul(p2[:, g, :], b2t[:, 128 * g:128 * g + 128],
                                         ones_t[:], start=True, stop=False)
                        nc.tensor.matmul(p2[:, g, :], w2t[:, 128 * g:128 * g + 128],
                                         rhs, start=False, stop=True)
                    nc.scalar.activation(
                        cur[j][:, :, 32:96].rearrange("p k (g b) -> p k g b", g=2),
                        p2[:].rearrange("p g (k b) -> p k g b", k=BLK),
                        mybir.ActivationFunctionType.Identity, bias=0.0, scale=0.5)

                def prep_c3(j):
                    # cur3 for block j from s2 of S[j-2], batch-half stacked:
                    # psum partitions 20h:20h+20 = features for batch half h
                    if j - 2 not in stk:
                        return
                    s = stk[j - 2]
                    p = pc3p.tile([F3S, BLK * NB2], f32, name="p_c3", tag="p_c3")
                    for hh in range(2):
                        ra = s[:, :, 32 + NB2 * hh:32 + NB2 * hh + NB2]
                        rb = s[:, :, 64 + NB2 * hh:64 + NB2 * hh + NB2]
                        nc.tensor.matmul(p[F3H * hh:F3H * hh + F3, :], w3ta[:], ra,
                                         start=True, stop=False)
                        nc.tensor.matmul(p[F3H * hh:F3H * hh + F3, :], w3tb[:], rb,
                                         start=False, stop=True)
                    nc.scalar.activation(
                        cur[j][0:F3S, :, 96:112],
                        p[:].rearrange("p (k b) -> p k b", k=BLK),
                        mybir.ActivationFunctionType.Identity, bias=b3[:], scale=0.5)

                def s3_accum(jj):
                    # park sum_k s3' of block jj in rts[:, jj-4, :]; one final
                    # reduce over blocks replaces 32 per-block accumulates.
                    t0 = BLK * (jj - 4)
                    kmax = min(BLK, T - t0)
                    if kmax <= 0 or jj not in s3k:
                        return
                    with nc.allow_low_precision("bf16 exact for +-1 sums up to 16"):
                        nc.vector.tensor_reduce(
                            rts[:, jj - 4, :], s3k[jj][:, :, 0:kmax],
                            mybir.AxisListType.X, AL.add)

                def extract(j):
                    # spikes of block j on ACT: sign(y - bound) in {-1,+1}
                    # (0 only at exact tie). Consumers correct affinely:
                    # s = (s' + 1)/2 via scale=0.5 + rowsum-adjusted biases.
                    h = (j % 2) * BLK
                    if j < NBLK - 2:
                        s_t = spool.tile([128, BLK, 96], bf16, name="sblk", tag="sblk")
                        nc.scalar.activation(
                            s_t[:], M[:, h:h + BLK, 0:96],
                            mybir.ActivationFunctionType.Sign,
                            bias=bndn[:], scale=1.0)
                        stk[j] = s_t
                        stk.pop(j - 3, None)
                    if j >= 4:
                        # s3 transposed (timestep innermost) for a packed reduce
                        s3t = s3pool.tile([F3S, NB2, BLK], bf16, name="s3t", tag="s3t")
                        nc.scalar.activation(
                            s3t[:], M[0:F3S, h:h + BLK, 96:112].rearrange("p k b -> p b k"),
                            mybir.ActivationFunctionType.Sign,
                            bias=bndn[0:F3S, :], scale=1.0)
                        s3k[j] = s3t
                        s3k.pop(j - 3, None)

                # ---- prologue: block 0 prep ----
                curc = None
                if "c1" in sk:
                    # chain-only ablation: constant current tile
                    curc = cpool.tile([128, BLK, FCOLS], bf16, name="curc", tag="curblk")
                    nc.gpsimd.memset(curc[:], idle)
                if "dma" not in sk:
                    dma_x(0)
                new_curblk(0)
                if "c1" not in sk:
                    prep_c1(0)

                for j in range(NBLK):
                    # prep cur[j+1] (runs during block j on PE/ACT/DMA)
                    if j + 1 < NBLK:
                        new_curblk(j + 1)
                        if j + 1 < XBLK:
                            if "dma" not in sk:
                                dma_x(j + 1)
                            if "c1" not in sk:
                                prep_c1(j + 1)
                        if 2 <= j + 1 and "c2" not in sk:
                            prep_c2(j + 1)
                        if 4 <= j + 1 and "c3" not in sk:
                            prep_c3(j + 1)
                    # s3 sum lagged 2 blocks: its extraction dep is long done
                    if j - 2 >= 4 and "s3" not in sk:
                        s3_accum(j - 2)

                    # serial LIF steps for block j (same-engine back-to-back)
                    h = (j % 2) * BLK
                    hp = ((j + 1) % 2) * BLK
                    if "chain" not in sk:
                        cj = curc if curc is not None else cur[j]
                        for k in range(BLK):
                            ysrc = M[:, hp + BLK - 1, :] if k == 0 else M[:, h + k - 1, :]
                            nc.vector._custom_dve(
                                LIF_YSTEP_ANT, out=M[:, h + k, :], in0=ysrc,
                                in1=cj[:, k, :], s0=beta, s1=bound, imm2=thr)
                    if "extract" not in sk:
                        extract(j)

                if "s3" not in sk:
                    s3_accum(NBLK - 2)
                    s3_accum(NBLK - 1)
                    # sum the 32 parked block-sums, then +T exactly (integer
                    # f32) so the no-spike case is 0 * scale = 0 exactly
                    nc.vector.tensor_reduce(
                        ssum[:], rts[:].rearrange("p j b -> p b j"),
                        mybir.AxisListType.X, AL.add)
                    nc.vector.tensor_scalar_add(ssum[:], ssum[:], float(T))
                nc.scalar.activation(o_tile[:], ssum[:],
                                     mybir.ActivationFunctionType.Identity,
                                     bias=0.0, scale=1.0 / (2.0 * T))
            nc.sync.dma_start(out=out_d[:, 0:NB2], in_=o_tile[0:F3, :])
            nc.sync.dma_start(out=out_d[:, NB2:NB], in_=o_tile[F3H:F3S, :])
    nc.compile()
    return nc


def stage_inputs(x, W1, b1, W2, b2, W3, b3, beta, thr):
    W2 = np.asarray(W2); W3 = np.asarray(W3)
    """Build per-core input maps (host-side sharding + layout + bf16 cast)."""
    in_maps = []
    W1p = np.zeros((CP, F1), dtype=np.float32)
    W1p[:C, :] = np.ascontiguousarray(W1.T)
    W1t = W1p.astype(ml_dtypes.bfloat16)                  # [768, 128]
    W2t = np.ascontiguousarray(W2.T).astype(ml_dtypes.bfloat16)   # [128, 256]
    W3t = np.ascontiguousarray(W3.T).astype(ml_dtypes.bfloat16)   # [256, 20]
    shift = np.float32((1.0 - beta) * BSHIFT)
    b1c = np.ascontiguousarray(b1.reshape(F1, 1).astype(np.float32) + shift)
    # spikes arrive as s' in {-1,+1} (ACT Sign); s = (s'+1)/2, so the
    # 0.5*rowsum(W) half lands in the bias and the matmul is scaled by 0.5.
    w2row = 0.5 * np.asarray(W2, np.float32).sum(axis=1).reshape(F2, 1)
    w3row = 0.5 * np.asarray(W3, np.float32).sum(axis=1).reshape(F3, 1)
    b2eff = b2.reshape(F2, 1).astype(np.float32) + shift + w2row
    b2c = np.ascontiguousarray((2.0 * b2eff).reshape(1, F2)).astype(ml_dtypes.bfloat16)
    b3one = b3.reshape(F3, 1).astype(np.float32) + shift + w3row
    b3c = np.zeros((F3S, 1), np.float32)
    b3c[0:F3] = b3one
    b3c[F3H:F3S] = b3one
    for c in range(NCORES):
        xc = x[c * NB:(c + 1) * NB]                        # [32, 500, 700]
        xT = np.transpose(xc, (2, 1, 0))                   # [700, 500, 32]
        Xp = np.zeros((CP, TP, NB), dtype=np.float32)
        Xp[:C, :T, :] = xT
        Xc = np.ascontiguousarray(Xp.reshape(CP, TP * NB)).astype(ml_dtypes.bfloat16)
        in_maps.append({
            "x": Xc, "w1t": W1t, "w2t": W2t, "w3t": W3t,
            "b1": b1c, "b2t": b2c, "b3": b3c,
        })
    return in_maps


_cache = {}
_last_result = None


def kernel(x, W1, b1, W2, b2, W3, b3,
           beta1, beta2, beta3, thr1, thr2, thr3):
    beta = float(np.clip(np.float32(beta1), 0.0, 1.0))
    thr = float(np.float32(thr1))
    assert float(beta2) == float(beta1) and float(beta3) == float(beta1)
    assert float(thr2) == float(thr1) and float(thr3) == float(thr1)

    key = (beta, thr)
    if key not in _cache:
        _cache[key] = build_kernel(beta, thr)
    nc = _cache[key]

    in_maps = stage_inputs(np.asarray(x, dtype=np.float32), np.asarray(W1), np.asarray(b1),
                           np.asarray(W2), np.asarray(b2), np.asarray(W3), np.asarray(b3),
                           beta, thr)
    res = run_bass_kernel_spmd(nc, in_maps, list(range(NCORES)))
    global _last_result
    _last_result = res
    out = np.zeros((B, F3), dtype=np.float32)
    for c in range(NCORES):
        out[c * NB:(c + 1) * NB, :] = res.results[c]["out"].T
    return out
